# revision 67
# baseline (speedup 1.0000x reference)
"""GQA attention kernel for 8 trn2 NeuronCores.

Sharding: core c in 0..7 -> batch b = c//4, KV group g = c%4 (4 Q heads,
1 KV head per core). Tensor-parallel on Wq/Wk/Wv columns and Wo rows;
host sums the 4 partial outputs per batch.

Precision: softmax-weight noise passes 1:1 to the output (the output is a
weighted mean, so its scale shrinks with the same sqrt(N) that averages the
noise). fp8 anywhere on the Q/K/exp path therefore fails the 2e-2 gate
(measured ~3e-2 per stage); the whole attention core runs bf16/fp16, which
lands ~3e-3. All matmuls run at 1 cycle/row (full PE rate).

Speed comes from engine balance and occupancy:
- softmax row-sums: DVE pairwise tree (fp16 2x/4x modes) + Pool level-2,
  one final ones-matmul pair on PE instead of 8 M=1 matmuls;
- PSUM->SBUF copies on Act (phase A) / DVE (phase C); Pool does the
  SBUF-side RoPE multiplies (it cannot touch PSUM);
- softmax tails and out-projection chunks are deferred and woven into the
  next head's S/exp stream so in-order PE never stalls on DVE latency.
"""
import sys
sys.path.insert(0, "/opt/trn_rl_repo")
import math
import numpy as np
import ml_dtypes

B, L, D = 2, 2048, 2048
H, HKV, HD = 16, 4, 128
BASE = 10000.0
NCH = L // 512     # 4 seq chunks of 512
NH = H // HKV      # 4 heads per core
WS = 32.0          # host prescale on W (keeps fp8-hi in the normal range);
                   # q,k carry x32 -> exp scale divides by 32*32; v carries
                   # x32 -> host divides y by 32
SCALE = 1.0 / (math.sqrt(HD) * WS * WS)

FP16 = np.float16
BF16 = ml_dtypes.bfloat16

_compiled = None


def _build():
    from concourse import bacc, tile, mybir

    f32, f32r = mybir.dt.float32, mybir.dt.float32r
    bf16, fp16 = mybir.dt.bfloat16, mybir.dt.float16
    Exp = mybir.ActivationFunctionType.Exp
    Copy = mybir.ActivationFunctionType.Copy
    mult, add, sub = (mybir.AluOpType.mult, mybir.AluOpType.add,
                      mybir.AluOpType.subtract)

    nc = bacc.Bacc("TRN2", target_bir_lowering=False, debug=False,
                   enable_asserts=True, num_devices=8)

    fp8, fp8e5 = mybir.dt.float8e4, mybir.dt.float8e5
    DR = mybir.MatmulPerfMode.DoubleRow
    x8h_d = nc.dram_tensor("x8h", [128, NCH, 8, 2, 512], fp8, kind="ExternalInput")
    x8l_d = nc.dram_tensor("x8l", [128, NCH, 8, 2, 512], fp8e5,
                           kind="ExternalInput")
    wq8h_d = nc.dram_tensor("wq8h", [128, 8, 2, 512], fp8, kind="ExternalInput")
    wq8l_d = nc.dram_tensor("wq8l", [128, 8, 2, 512], fp8e5, kind="ExternalInput")
    wk8h_d = nc.dram_tensor("wk8h", [128, 8, 2, 128], fp8, kind="ExternalInput")
    wk8l_d = nc.dram_tensor("wk8l", [128, 8, 2, 128], fp8e5, kind="ExternalInput")
    wv8h_d = nc.dram_tensor("wv8h", [128, 8, 2, 128], fp8, kind="ExternalInput")
    wv8l_d = nc.dram_tensor("wv8l", [128, 8, 2, 128], fp8e5, kind="ExternalInput")
    wo8h_d = nc.dram_tensor("wo8h", [2, 128, 2, D], fp8, kind="ExternalInput")
    wo8l_d = nc.dram_tensor("wo8l", [2, 128, 2, D], fp8e5, kind="ExternalInput")
    cos_d = nc.dram_tensor("cosT", [HD, L], bf16, kind="ExternalInput")
    sin_d = nc.dram_tensor("sinT", [HD, L], bf16, kind="ExternalInput")
    shp_d = nc.dram_tensor("shiftP", [HD, HD], bf16, kind="ExternalInput")
    idn_d = nc.dram_tensor("ident", [128, 128], bf16, kind="ExternalInput")
    y_d = nc.dram_tensor("y", [L, D], f32, kind="ExternalOutput")

    with tile.TileContext(nc) as tc, \
         nc.allow_low_precision(reason="bf16/fp16 attention core; see module "
                                "docstring noise analysis"):
        with tc.tile_pool(name="persist", bufs=1) as pp:
            qt = [[pp.tile([HD, 512], bf16, tag=f"qt{h}_{n}", name=f"qt{h}_{n}")
                   for n in range(NCH)] for h in range(NH)]
            kt = [pp.tile([HD, 512], bf16, tag=f"kt{n}", name=f"kt{n}")
                  for n in range(NCH)]
            vn = [pp.tile([128, HD], fp16, tag=f"vn{t}", name=f"vn{t}")
                  for t in range(16)]
            # attention outputs in hi/lo fp8, head-PAIRED on dim 1 for the
            # DoubleRow out-projection
            oth = [[pp.tile([HD, 2, 512], fp8, tag=f"oth{u}_{n}",
                            name=f"oth{u}_{n}") for n in range(NCH)]
                   for u in range(2)]
            otl = [[pp.tile([HD, 2, 512], fp8e5, tag=f"otl{u}_{n}",
                            name=f"otl{u}_{n}") for n in range(NCH)]
                   for u in range(2)]
            woh = [pp.tile([HD, 2, L], fp8, tag=f"woh{u}", name=f"woh{u}")
                   for u in range(2)]
            wol = [pp.tile([HD, 2, L], fp8e5, tag=f"wol{u}", name=f"wol{u}")
                   for u in range(2)]
            shp = pp.tile([HD, HD], bf16, tag="shp", name="shp")
            idn = pp.tile([128, 128], bf16, tag="idn", name="idn")
            cosT = pp.tile([HD, L], bf16, tag="cos", name="cos")
            sinT = pp.tile([HD, L], bf16, tag="sin", name="sin")

            # ---------------- Phase A: projections + RoPE + V transpose ------
            with tc.tile_pool(name="aw", bufs=1) as aw, \
                 tc.tile_pool(name="ax", bufs=1) as ax, \
                 tc.tile_pool(name="atmp", bufs=3) as at, \
                 tc.tile_pool(name="apsum", bufs=1, space="PSUM") as aps:
                wqh = aw.tile([128, 8, 2, 512], fp8, tag="wqh", name="wqh")
                wql = aw.tile([128, 8, 2, 512], fp8e5, tag="wql", name="wql")
                wkh = aw.tile([128, 8, 2, 128], fp8, tag="wkh", name="wkh")
                wkl = aw.tile([128, 8, 2, 128], fp8e5, tag="wkl", name="wkl")
                wvh = aw.tile([128, 8, 2, 128], fp8, tag="wvh", name="wvh")
                wvl = aw.tile([128, 8, 2, 128], fp8e5, tag="wvl", name="wvl")
                xhs = [ax.tile([128, 8, 2, 512], fp8, tag=f"xh{i}", name=f"xh{i}")
                       for i in range(2)]
                xls = [ax.tile([128, 8, 2, 512], fp8e5, tag=f"xl{i}",
                               name=f"xl{i}") for i in range(2)]
                # DMA issue order: what the first matmuls need, first.
                # (single SP queue executes in order; wo waits until phase B)
                for sl in (slice(0, 2), slice(2, 4), slice(4, 6),
                           slice(6, 8)):
                    nc.sync.dma_start(wqh[:, sl], wq8h_d[:, sl])
                    nc.sync.dma_start(xhs[0][:, sl], x8h_d[:, 0, sl])
                    nc.sync.dma_start(xls[0][:, sl], x8l_d[:, 0, sl])
                    nc.sync.dma_start(wql[:, sl], wq8l_d[:, sl])
                    nc.sync.dma_start(wkh[:, sl], wk8h_d[:, sl])
                    nc.sync.dma_start(wkl[:, sl], wk8l_d[:, sl])
                    nc.sync.dma_start(wvh[:, sl], wv8h_d[:, sl])
                    nc.sync.dma_start(wvl[:, sl], wv8l_d[:, sl])
                nc.sync.dma_start(idn[:], idn_d[:])
                for n in range(NCH):
                    ps = [aps.tile([128, 512], f32, tag=f"pa{j}", name=f"pa{j}")
                          for j in range(6)]
                    xh, xl = xhs[n % 2], xls[n % 2]
                    if n + 1 < NCH:   # prefetch next chunk
                        for q in range(4):
                            sl = slice(2 * q, 2 * (q + 1))
                            nc.sync.dma_start(xhs[(n + 1) % 2][:, sl],
                                              x8h_d[:, n + 1, sl])
                            nc.sync.dma_start(xls[(n + 1) % 2][:, sl],
                                              x8l_d[:, n + 1, sl])
                    if n == 0:
                        # needed only from the chunk-0 RoPE onward; issued
                        # after the chunk-1 prefetch so that isn't delayed
                        nc.sync.dma_start(cosT[:], cos_d[:])
                        nc.sync.dma_start(sinT[:], sin_d[:])
                        nc.sync.dma_start(shp[:], shp_d[:])
                        # preload the Exp table while Act is idle so the
                        # first phase-B exp doesn't pay LoadActFuncSet
                        warm = at.tile([1, 8], f32, tag="warm", name="warm")
                        nc.scalar.activation(warm[:], shp[0:1, 0:8], Exp)
                    # hi/lo fp8 DoubleRow: exact - (x_lo @ w_lo); contraction
                    # pairs c-tiles (2t, 2t+1) on dim 1. At t=7, v/k groups
                    # stop first so their PSUM->SBUF copies start ASAP.
                    for t in range(8):
                        kv = ((5, wvh, wvl), (4, wkh, wkl))
                        for pj, wh_, wl_ in (kv if t == 7 else ()):
                            for wt, xt in ((wh_, xh), (wh_, xl), (wl_, xh)):
                                nc.tensor.matmul(
                                    ps[pj][:], wt[:, t], xt[:, t],
                                    perf_mode=DR, start=False,
                                    stop=(wt is wl_))
                        for j in range(NH):
                            js = slice(j * 128, (j + 1) * 128)
                            for wt, xt in ((wqh, xh), (wqh, xl), (wql, xh)):
                                nc.tensor.matmul(
                                    ps[j][:], wt[:, t, :, js], xt[:, t],
                                    perf_mode=DR,
                                    start=(t == 0 and xt is xh and wt is wqh),
                                    stop=(t == 7 and wt is wql))
                        if t < 7:
                            for pj, wh_, wl_ in kv:
                                for wt, xt in ((wh_, xh), (wh_, xl), (wl_, xh)):
                                    nc.tensor.matmul(
                                        ps[pj][:], wt[:, t], xt[:, t],
                                        perf_mode=DR,
                                        start=(t == 0 and xt is xh
                                               and wt is wh_),
                                        stop=False)
                    cs = cosT[:, n * 512:(n + 1) * 512]
                    sn = sinT[:, n * 512:(n + 1) * 512]
                    # vf first: PE's next work (transposes) depends on it;
                    # k's RoPE first: phase B's S matmuls depend on kt
                    vf = at.tile([128, 512], bf16, tag="vf", name="vf")
                    nc.scalar.activation(vf[:], ps[5][:], Copy)
                    raws = {}
                    for j in (4, 0, 1, 2, 3):
                        raw = at.tile([128, 512], bf16, tag=f"raw{j}",
                                      name=f"raw{j}", bufs=2)
                        nc.scalar.activation(raw[:], ps[j][:], Copy)
                        # roll along HD via partition-shifted SBUF copies
                        rol = at.tile([128, 512], bf16, tag=f"rol{j}",
                                      name=f"rol{j}", bufs=2)
                        nc.sync.dma_start(rol[1:128, :], raw[0:127, :])
                        nc.sync.dma_start(rol[0:1, :], raw[127:128, :])
                        raws[j] = (raw, rol)
                        if j == 4:
                            for t in range(4):
                                pvt = aps.tile([128, 128], bf16, tag="pvt",
                                               name="pvt")
                                nc.tensor.transpose(
                                    pvt[:], vf[:, t * 128:(t + 1) * 128], idn[:])
                                nc.vector.tensor_copy(vn[n * 4 + t][:], pvt[:])
                    for j in (4, 0, 1, 2, 3):
                        raw, rol = raws[j]
                        t1 = at.tile([128, 512], bf16, tag="t1", name="t1")
                        nc.gpsimd.tensor_tensor(t1[:], raw[:], cs, mult)
                        t2 = at.tile([128, 512], bf16, tag="t2", name="t2")
                        nc.vector.tensor_tensor(t2[:], rol[:], sn, mult)
                        dst = qt[j][n] if j < NH else kt[n]
                        nc.vector.tensor_tensor(dst[:], t1[:], t2[:], add)

            # ------------- Phase B: attention; Phase C: out-projection -------
            # Deferred-work queue: softmax tails and out-projection chunks are
            # emitted interleaved with later heads' S/exp stream so the
            # in-order PE queue never waits on the DVE/Pool reduction chain.
            with tc.tile_pool(name="bexp", bufs=6) as bx, \
                 tc.tile_pool(name="bsacc", bufs=3) as bsa, \
                 tc.tile_pool(name="bsm", bufs=2) as bs, \
                 tc.tile_pool(name="yout", bufs=4) as yp, \
                 tc.tile_pool(name="bpsum", bufs=1, space="PSUM") as bps, \
                 tc.tile_pool(name="cpsum", bufs=2, space="PSUM") as cps:
                workq = []

                def pump(k):
                    for _ in range(min(k, len(workq))):
                        workq.pop(0)()

                from concourse import bass_isa
                for u in range(2):
                    nc.sync.dma_start(woh[u][:], wo8h_d[u])
                    nc.sync.dma_start(wol[u][:], wo8l_d[u])

                def mk_tail(h, qb, saccs, pso):
                    def tail():
                        # all-DVE tree to one [128,512] tile, then a gpsimd
                        # partition all-reduce gives every partition the row
                        # sum -- no PSUM, no ones-matmul, no broadcast matmul
                        l2a = bsa.tile([128, 2, 512], fp16, tag="l2a", name="l2a")
                        nc.vector.tensor_tensor(l2a[:], saccs[0][:], saccs[1][:],
                                                add)
                        l2b = bsa.tile([128, 2, 512], fp16, tag="l2b", name="l2b")
                        nc.vector.tensor_tensor(l2b[:], saccs[2][:], saccs[3][:],
                                                add)
                        l3 = bsa.tile([128, 2, 512], fp16, tag="l3", name="l3")
                        nc.vector.tensor_tensor(l3[:], l2a[:], l2b[:], add)
                        sht = bsa.tile([128, 512], f32, tag="sht", name="sht")
                        nc.vector.tensor_tensor(sht[:], l3[:, 0, :], l3[:, 1, :],
                                                add)
                        sums = bsa.tile([128, 512], f32, tag="sums", name="sums")
                        nc.gpsimd.partition_all_reduce(sums[:], sht[:], 128,
                                                       bass_isa.ReduceOp.add)
                        rec = bs.tile([128, 512], f32r, tag="rec", name="rec")
                        nc.vector.reciprocal(rec[:], sums[:])
                        nf = bs.tile([128, 512], f32, tag="nf", name="nf")
                        nc.vector.tensor_tensor(nf[:], pso[:], rec[:], mult)
                        u, i = h // 2, h % 2
                        # last qb: DVE for the hi/lo split -- it is on the
                        # critical chain into the final out-projection drain
                        eng = nc.vector if qb == NCH - 1 else nc.gpsimd
                        eng.tensor_copy(oth[u][qb][:, i, :], nf[:])
                        eng.tensor_tensor(otl[u][qb][:, i, :], nf[:],
                                          oth[u][qb][:, i, :], sub)
                    return tail

                def mk_cchunk(qb, ti, nn, ysb, last):
                    def cchunk(u=None):
                        # u=None: both head-pairs in one psum group.
                        # u=0/1: split passes (last qb) -- pair-0 matmuls can
                        # run while pair-1's softmax tail is still finishing.
                        ts_ = slice(ti * 128, (ti + 1) * 128)
                        ns_ = slice(nn * 512, (nn + 1) * 512)
                        ys = ysb[:, nn * 512:(nn + 1) * 512]
                        us = (0, 1) if u is None else (u,)
                        psy = cps.tile([128, 512], f32, tag="psy", name="psy")
                        for uu in us:
                            for m, (a, w) in enumerate(
                                    ((oth, woh), (oth, wol), (otl, woh))):
                                nc.tensor.matmul(
                                    psy[:], a[uu][qb][:, :, ts_], w[uu][:, :, ns_],
                                    perf_mode=DR,
                                    start=(uu == us[0] and m == 0),
                                    stop=(uu == us[-1] and m == 2))
                        if u == 0:
                            nc.vector.tensor_copy(ys, psy[:])
                            return
                        if u == 1:
                            nc.vector.tensor_tensor(ys, ys, psy[:], add)
                        elif last and nn % 2 == 0:
                            nc.scalar.activation(ys, psy[:], Copy)
                        else:
                            nc.vector.tensor_copy(ys, psy[:])
                        qtile = qb * 4 + ti
                        if last:
                            # final tile: store per-slice so the tail DMA is
                            # small and the drain starts sooner
                            nc.sync.dma_start(
                                y_d[qtile * 128:(qtile + 1) * 128,
                                    nn * 512:(nn + 1) * 512], ys)
                        elif nn == NCH - 1:
                            nc.sync.dma_start(
                                y_d[qtile * 128:(qtile + 1) * 128, :], ysb[:])
                    return cchunk

                for qb in range(NCH):
                    for h in range(NH):
                        pso = bps.tile([128, 512], f32, tag=f"pso{(qb * 4 + h) % 2}",
                                       name="pso")
                        saccs, es_tiles = [], []

                        def consume(pt):
                            # AV + level-1 row-sum for es_tiles[pt], one step
                            # behind the S/exp stream so PE never waits on Act
                            es = es_tiles[pt]
                            nc.tensor.matmul(pso[:], vn[2 * pt][:], es[:, 0, :],
                                             start=(pt == 0), stop=False)
                            nc.tensor.matmul(pso[:], vn[2 * pt + 1][:],
                                             es[:, 1, :],
                                             start=False, stop=(pt == 7))
                            if pt % 2 == 1:
                                sa = bsa.tile([128, 2, 512], fp16,
                                              tag=f"sa{pt // 2}",
                                              name=f"sa{pt // 2}")
                                nc.vector.tensor_tensor(sa[:], es_tiles[pt - 1][:],
                                                        es[:], add)
                                saccs.append(sa)

                        for pt in range(8):
                            pss = bps.tile([128, 2, 512], f32,
                                           tag=f"pss{pt % 2}", name=f"pss{pt % 2}")
                            for half in range(2):
                                k = 2 * pt + half
                                nc.tensor.matmul(
                                    pss[:, half, :],
                                    kt[k // 4][:, (k % 4) * 128:(k % 4 + 1) * 128],
                                    qt[h][qb][:], start=True, stop=True)
                            es = bx.tile([128, 2, 512], fp16, tag="es", name="es")
                            nc.scalar.activation(es[:], pss[:], Exp, scale=SCALE)
                            es_tiles.append(es)
                            if pt > 0:
                                consume(pt - 1)
                            pump(1)
                        consume(7)
                        workq.append(mk_tail(h, qb, saccs, pso))
                    # let the last head's tail chain (~5us of DVE/Pool
                    # latency) finish before the first C chunk needs its ot
                    workq.extend([lambda: None] * 4)
                    for ti in range(4):
                        ysb = yp.tile([128, L], f32, tag="ysb", name="ysb")
                        for nn in range(NCH):
                            workq.append(mk_cchunk(qb, ti, nn, ysb,
                                                   qb == NCH - 1))
                pump(len(workq))

    nc.compile()
    return nc


def _host_inputs(x, Wq, Wk, Wv, Wo):
    inv = 1.0 / (BASE ** (np.arange(0, HD, 2, dtype=np.float32) / HD))
    pos = np.arange(L, dtype=np.float32)
    fr = pos[:, None] * inv[None, :]
    emb = np.concatenate([fr, fr], axis=1)            # [L, HD]
    cosT = np.ascontiguousarray(np.cos(emb).T).astype(BF16)
    sinT = np.ascontiguousarray(np.sin(emb).T).astype(BF16)
    shp = np.zeros((HD, HD), np.float32)
    shp[(np.arange(HD) - 1) % HD, np.arange(HD)] = 1.0
    idn = np.eye(128, dtype=np.float32)

    FP8 = ml_dtypes.float8_e4m3
    FP8E5 = ml_dtypes.float8_e5m2

    def hilo(a):
        hi = a.astype(FP8)
        lo = (a - hi.astype(np.float32)).astype(FP8E5)
        return hi, lo

    maps = []
    for c in range(8):
        b, g = c // 4, c % 4
        xT = x[b].T                                    # [D, L]
        # [p, n, t, i, m] = xT[256t+128i+p, 512n+m]
        xa = xT.reshape(8, 2, 128, NCH, 512).transpose(2, 3, 0, 1, 4)
        x8h, x8l = hilo(np.ascontiguousarray(xa))
        # weights: [p, t, i, m] = 32*W[256t+128i+p, m]
        wq = (Wq[:, g * NH * HD:(g + 1) * NH * HD] * WS)
        wq8h, wq8l = hilo(np.ascontiguousarray(
            wq.reshape(8, 2, 128, NH * HD).transpose(2, 0, 1, 3)))
        wk = (Wk[:, g * HD:(g + 1) * HD] * WS)
        wk8h, wk8l = hilo(np.ascontiguousarray(
            wk.reshape(8, 2, 128, HD).transpose(2, 0, 1, 3)))
        wv = (Wv[:, g * HD:(g + 1) * HD] * WS)
        wv8h, wv8l = hilo(np.ascontiguousarray(
            wv.reshape(8, 2, 128, HD).transpose(2, 0, 1, 3)))
        # wo pairs: [u, p, i, m] = 32*Wo[g*512 + (2u+i)*128 + p, m]
        wo = (Wo[g * NH * HD:(g + 1) * NH * HD, :] * WS)
        wo8h, wo8l = hilo(np.ascontiguousarray(
            wo.reshape(2, 2, 128, D).transpose(0, 2, 1, 3)))
        maps.append({
            "x8h": x8h, "x8l": x8l,
            "wq8h": wq8h, "wq8l": wq8l, "wk8h": wk8h, "wk8l": wk8l,
            "wv8h": wv8h, "wv8l": wv8l, "wo8h": wo8h, "wo8l": wo8l,
            "cosT": cosT, "sinT": sinT,
            "shiftP": shp.astype(BF16), "ident": idn.astype(BF16),
        })
    return maps


def _run(inputs, trace=False):
    global _compiled
    from concourse.bass_utils import run_bass_kernel_spmd
    if _compiled is None:
        _compiled = _build()
    maps = _host_inputs(inputs["x"], inputs["Wq"], inputs["Wk"],
                        inputs["Wv"], inputs["Wo"])
    res = run_bass_kernel_spmd(_compiled, maps, list(range(8)), trace=trace)
    y = np.empty((B, L, D), np.float32)
    for b in range(B):
        y[b] = res.results[b * 4]["y"]
        for g in range(1, 4):
            y[b] += res.results[b * 4 + g]["y"]
    y *= 1.0 / (WS * WS)   # v and wo each carry the x32 host prescale
    return y, res


def kernel(**inputs):
    x = np.asarray(inputs["x"], np.float32)
    y, _ = _run({"x": x,
                 "Wq": np.asarray(inputs["Wq"], np.float32),
                 "Wk": np.asarray(inputs["Wk"], np.float32),
                 "Wv": np.asarray(inputs["Wv"], np.float32),
                 "Wo": np.asarray(inputs["Wo"], np.float32)})
    return y


# revision 68
# speedup vs baseline: 1.0020x; 1.0020x over previous
"""GQA attention kernel for 8 trn2 NeuronCores.

Sharding: core c in 0..7 -> batch b = c//4, KV group g = c%4 (4 Q heads,
1 KV head per core). Tensor-parallel on Wq/Wk/Wv columns and Wo rows;
host sums the 4 partial outputs per batch.

Precision: softmax-weight noise passes 1:1 to the output (the output is a
weighted mean, so its scale shrinks with the same sqrt(N) that averages the
noise). fp8 anywhere on the Q/K/exp path therefore fails the 2e-2 gate
(measured ~3e-2 per stage); the whole attention core runs bf16/fp16, which
lands ~3e-3. All matmuls run at 1 cycle/row (full PE rate).

Speed comes from engine balance and occupancy:
- softmax row-sums: DVE pairwise tree (fp16 2x/4x modes) + Pool level-2,
  one final ones-matmul pair on PE instead of 8 M=1 matmuls;
- PSUM->SBUF copies on Act (phase A) / DVE (phase C); Pool does the
  SBUF-side RoPE multiplies (it cannot touch PSUM);
- softmax tails and out-projection chunks are deferred and woven into the
  next head's S/exp stream so in-order PE never stalls on DVE latency.
"""
import sys
sys.path.insert(0, "/opt/trn_rl_repo")
import math
import numpy as np
import ml_dtypes

B, L, D = 2, 2048, 2048
H, HKV, HD = 16, 4, 128
BASE = 10000.0
NCH = L // 512     # 4 seq chunks of 512
NH = H // HKV      # 4 heads per core
WS = 32.0          # host prescale on W (keeps fp8-hi in the normal range);
                   # q,k carry x32 -> exp scale divides by 32*32; v carries
                   # x32 -> host divides y by 32
SCALE = 1.0 / (math.sqrt(HD) * WS * WS)

FP16 = np.float16
BF16 = ml_dtypes.bfloat16

_compiled = None


def _build():
    from concourse import bacc, tile, mybir

    f32, f32r = mybir.dt.float32, mybir.dt.float32r
    bf16, fp16 = mybir.dt.bfloat16, mybir.dt.float16
    Exp = mybir.ActivationFunctionType.Exp
    Copy = mybir.ActivationFunctionType.Copy
    mult, add, sub = (mybir.AluOpType.mult, mybir.AluOpType.add,
                      mybir.AluOpType.subtract)

    nc = bacc.Bacc("TRN2", target_bir_lowering=False, debug=False,
                   enable_asserts=True, num_devices=8)

    fp8, fp8e5 = mybir.dt.float8e4, mybir.dt.float8e5
    DR = mybir.MatmulPerfMode.DoubleRow
    x8h_d = nc.dram_tensor("x8h", [128, NCH, 8, 2, 512], fp8, kind="ExternalInput")
    x8l_d = nc.dram_tensor("x8l", [128, NCH, 8, 2, 512], fp8e5,
                           kind="ExternalInput")
    wq8h_d = nc.dram_tensor("wq8h", [128, 8, 2, 512], fp8, kind="ExternalInput")
    wq8l_d = nc.dram_tensor("wq8l", [128, 8, 2, 512], fp8e5, kind="ExternalInput")
    wk8h_d = nc.dram_tensor("wk8h", [128, 8, 2, 128], fp8, kind="ExternalInput")
    wk8l_d = nc.dram_tensor("wk8l", [128, 8, 2, 128], fp8e5, kind="ExternalInput")
    wv8h_d = nc.dram_tensor("wv8h", [128, 8, 2, 128], fp8, kind="ExternalInput")
    wv8l_d = nc.dram_tensor("wv8l", [128, 8, 2, 128], fp8e5, kind="ExternalInput")
    wo8h_d = nc.dram_tensor("wo8h", [2, 128, 2, D], fp8, kind="ExternalInput")
    wo8l_d = nc.dram_tensor("wo8l", [2, 128, 2, D], fp8e5, kind="ExternalInput")
    cos_d = nc.dram_tensor("cosT", [HD, L], bf16, kind="ExternalInput")
    sin_d = nc.dram_tensor("sinT", [HD, L], bf16, kind="ExternalInput")
    shp_d = nc.dram_tensor("shiftP", [HD, HD], bf16, kind="ExternalInput")
    idn_d = nc.dram_tensor("ident", [128, 128], bf16, kind="ExternalInput")
    y_d = nc.dram_tensor("y", [L, D], f32, kind="ExternalOutput")

    with tile.TileContext(nc) as tc, \
         nc.allow_low_precision(reason="bf16/fp16 attention core; see module "
                                "docstring noise analysis"):
        with tc.tile_pool(name="persist", bufs=1) as pp:
            qt = [[pp.tile([HD, 512], bf16, tag=f"qt{h}_{n}", name=f"qt{h}_{n}")
                   for n in range(NCH)] for h in range(NH)]
            kt = [pp.tile([HD, 512], bf16, tag=f"kt{n}", name=f"kt{n}")
                  for n in range(NCH)]
            vn = [pp.tile([128, HD], fp16, tag=f"vn{t}", name=f"vn{t}")
                  for t in range(16)]
            # attention outputs in hi/lo fp8, head-PAIRED on dim 1 for the
            # DoubleRow out-projection
            oth = [[pp.tile([HD, 2, 512], fp8, tag=f"oth{u}_{n}",
                            name=f"oth{u}_{n}") for n in range(NCH)]
                   for u in range(2)]
            otl = [[pp.tile([HD, 2, 512], fp8e5, tag=f"otl{u}_{n}",
                            name=f"otl{u}_{n}") for n in range(NCH)]
                   for u in range(2)]
            woh = [pp.tile([HD, 2, L], fp8, tag=f"woh{u}", name=f"woh{u}")
                   for u in range(2)]
            wol = [pp.tile([HD, 2, L], fp8e5, tag=f"wol{u}", name=f"wol{u}")
                   for u in range(2)]
            shp = pp.tile([HD, HD], bf16, tag="shp", name="shp")
            idn = pp.tile([128, 128], bf16, tag="idn", name="idn")
            cosT = pp.tile([HD, L], bf16, tag="cos", name="cos")
            sinT = pp.tile([HD, L], bf16, tag="sin", name="sin")

            # ---------------- Phase A: projections + RoPE + V transpose ------
            with tc.tile_pool(name="aw", bufs=1) as aw, \
                 tc.tile_pool(name="ax", bufs=1) as ax, \
                 tc.tile_pool(name="atmp", bufs=3) as at, \
                 tc.tile_pool(name="apsum", bufs=1, space="PSUM") as aps:
                wqh = aw.tile([128, 8, 2, 512], fp8, tag="wqh", name="wqh")
                wql = aw.tile([128, 8, 2, 512], fp8e5, tag="wql", name="wql")
                wkh = aw.tile([128, 8, 2, 128], fp8, tag="wkh", name="wkh")
                wkl = aw.tile([128, 8, 2, 128], fp8e5, tag="wkl", name="wkl")
                wvh = aw.tile([128, 8, 2, 128], fp8, tag="wvh", name="wvh")
                wvl = aw.tile([128, 8, 2, 128], fp8e5, tag="wvl", name="wvl")
                xhs = [ax.tile([128, 8, 2, 512], fp8, tag=f"xh{i}", name=f"xh{i}")
                       for i in range(2)]
                xls = [ax.tile([128, 8, 2, 512], fp8e5, tag=f"xl{i}",
                               name=f"xl{i}") for i in range(2)]
                # DMA issue order: what the first matmuls need, first.
                # (single SP queue executes in order; wo waits until phase B)
                for sl in (slice(0, 2), slice(2, 4), slice(4, 6),
                           slice(6, 8)):
                    nc.sync.dma_start(wqh[:, sl], wq8h_d[:, sl])
                    nc.sync.dma_start(xhs[0][:, sl], x8h_d[:, 0, sl])
                    nc.sync.dma_start(xls[0][:, sl], x8l_d[:, 0, sl])
                    nc.sync.dma_start(wql[:, sl], wq8l_d[:, sl])
                    nc.sync.dma_start(wkh[:, sl], wk8h_d[:, sl])
                    nc.sync.dma_start(wkl[:, sl], wk8l_d[:, sl])
                    nc.sync.dma_start(wvh[:, sl], wv8h_d[:, sl])
                    nc.sync.dma_start(wvl[:, sl], wv8l_d[:, sl])
                nc.sync.dma_start(idn[:], idn_d[:])
                for n in range(NCH):
                    ps = [aps.tile([128, 512], f32, tag=f"pa{j}", name=f"pa{j}")
                          for j in range(6)]
                    xh, xl = xhs[n % 2], xls[n % 2]
                    if n + 1 < NCH:   # prefetch next chunk
                        for q in range(4):
                            sl = slice(2 * q, 2 * (q + 1))
                            nc.sync.dma_start(xhs[(n + 1) % 2][:, sl],
                                              x8h_d[:, n + 1, sl])
                            nc.sync.dma_start(xls[(n + 1) % 2][:, sl],
                                              x8l_d[:, n + 1, sl])
                    if n == 0:
                        # needed only from the chunk-0 RoPE onward; issued
                        # after the chunk-1 prefetch so that isn't delayed
                        nc.sync.dma_start(cosT[:], cos_d[:])
                        nc.sync.dma_start(sinT[:], sin_d[:])
                        nc.sync.dma_start(shp[:], shp_d[:])
                        # preload the Exp table while Act is idle so the
                        # first phase-B exp doesn't pay LoadActFuncSet
                        warm = at.tile([1, 8], f32, tag="warm", name="warm")
                        nc.scalar.activation(warm[:], shp[0:1, 0:8], Exp)
                    # hi/lo fp8 DoubleRow: exact - (x_lo @ w_lo); contraction
                    # pairs c-tiles (2t, 2t+1) on dim 1. At t=7, v/k groups
                    # stop first so their PSUM->SBUF copies start ASAP.
                    for t in range(8):
                        kv = ((5, wvh, wvl), (4, wkh, wkl))
                        for pj, wh_, wl_ in (kv if t == 7 else ()):
                            for wt, xt in ((wh_, xh), (wh_, xl), (wl_, xh)):
                                nc.tensor.matmul(
                                    ps[pj][:], wt[:, t], xt[:, t],
                                    perf_mode=DR, start=False,
                                    stop=(wt is wl_))
                        for j in range(NH):
                            js = slice(j * 128, (j + 1) * 128)
                            for wt, xt in ((wqh, xh), (wqh, xl), (wql, xh)):
                                nc.tensor.matmul(
                                    ps[j][:], wt[:, t, :, js], xt[:, t],
                                    perf_mode=DR,
                                    start=(t == 0 and xt is xh and wt is wqh),
                                    stop=(t == 7 and wt is wql))
                        if t < 7:
                            for pj, wh_, wl_ in kv:
                                for wt, xt in ((wh_, xh), (wh_, xl), (wl_, xh)):
                                    nc.tensor.matmul(
                                        ps[pj][:], wt[:, t], xt[:, t],
                                        perf_mode=DR,
                                        start=(t == 0 and xt is xh
                                               and wt is wh_),
                                        stop=False)
                    cs = cosT[:, n * 512:(n + 1) * 512]
                    sn = sinT[:, n * 512:(n + 1) * 512]
                    # vf first: PE's next work (transposes) depends on it;
                    # k's RoPE first: phase B's S matmuls depend on kt
                    vf = at.tile([128, 512], bf16, tag="vf", name="vf")
                    nc.scalar.activation(vf[:], ps[5][:], Copy)
                    raws = {}
                    for j in (4, 0, 1, 2, 3):
                        raw = at.tile([128, 512], bf16, tag=f"raw{j}",
                                      name=f"raw{j}", bufs=2)
                        nc.scalar.activation(raw[:], ps[j][:], Copy)
                        # roll along HD via partition-shifted SBUF copies
                        rol = at.tile([128, 512], bf16, tag=f"rol{j}",
                                      name=f"rol{j}", bufs=2)
                        nc.sync.dma_start(rol[1:128, :], raw[0:127, :])
                        nc.sync.dma_start(rol[0:1, :], raw[127:128, :])
                        raws[j] = (raw, rol)
                        if j == 4:
                            for t in range(4):
                                pvt = aps.tile([128, 128], bf16, tag="pvt",
                                               name="pvt")
                                nc.tensor.transpose(
                                    pvt[:], vf[:, t * 128:(t + 1) * 128], idn[:])
                                nc.vector.tensor_copy(vn[n * 4 + t][:], pvt[:])
                    for j in (4, 0, 1, 2, 3):
                        raw, rol = raws[j]
                        t1 = at.tile([128, 512], bf16, tag="t1", name="t1")
                        nc.gpsimd.tensor_tensor(t1[:], raw[:], cs, mult)
                        t2 = at.tile([128, 512], bf16, tag="t2", name="t2")
                        nc.vector.tensor_tensor(t2[:], rol[:], sn, mult)
                        dst = qt[j][n] if j < NH else kt[n]
                        nc.vector.tensor_tensor(dst[:], t1[:], t2[:], add)

            # ------------- Phase B: attention; Phase C: out-projection -------
            # Deferred-work queue: softmax tails and out-projection chunks are
            # emitted interleaved with later heads' S/exp stream so the
            # in-order PE queue never waits on the DVE/Pool reduction chain.
            with tc.tile_pool(name="bexp", bufs=6) as bx, \
                 tc.tile_pool(name="bsacc", bufs=2) as bsa, \
                 tc.tile_pool(name="bsm", bufs=2) as bs, \
                 tc.tile_pool(name="yout", bufs=4) as yp, \
                 tc.tile_pool(name="bpsum", bufs=1, space="PSUM") as bps, \
                 tc.tile_pool(name="cpsum", bufs=2, space="PSUM") as cps:
                workq = []

                def pump(k):
                    for _ in range(min(k, len(workq))):
                        workq.pop(0)()

                from concourse import bass_isa
                for u in range(2):
                    nc.sync.dma_start(woh[u][:], wo8h_d[u])
                    nc.sync.dma_start(wol[u][:], wo8l_d[u])

                def mk_tail(h, qb, saccs, pso):
                    def tail():
                        # all-DVE tree to one [128,512] tile, then a gpsimd
                        # partition all-reduce gives every partition the row
                        # sum -- no PSUM, no ones-matmul, no broadcast matmul
                        l2a = bsa.tile([128, 2, 512], fp16, tag="l2a", name="l2a")
                        nc.vector.tensor_tensor(l2a[:], saccs[0][:], saccs[1][:],
                                                add)
                        l2b = bsa.tile([128, 2, 512], fp16, tag="l2b", name="l2b")
                        nc.vector.tensor_tensor(l2b[:], saccs[2][:], saccs[3][:],
                                                add)
                        l3 = bsa.tile([128, 2, 512], fp16, tag="l3", name="l3")
                        nc.vector.tensor_tensor(l3[:], l2a[:], l2b[:], add)
                        sht = bsa.tile([128, 512], f32, tag="sht", name="sht")
                        nc.vector.tensor_tensor(sht[:], l3[:, 0, :], l3[:, 1, :],
                                                add)
                        sums = bsa.tile([128, 512], f32, tag="sums", name="sums")
                        nc.gpsimd.partition_all_reduce(sums[:], sht[:], 128,
                                                       bass_isa.ReduceOp.add)
                        rec = bs.tile([128, 512], f32r, tag="rec", name="rec")
                        nc.vector.reciprocal(rec[:], sums[:])
                        nf = bs.tile([128, 512], f32, tag="nf", name="nf")
                        nc.vector.tensor_tensor(nf[:], pso[:], rec[:], mult)
                        u, i = h // 2, h % 2
                        # last qb: DVE for the hi/lo split -- it is on the
                        # critical chain into the final out-projection drain
                        eng = nc.vector if qb == NCH - 1 else nc.gpsimd
                        eng.tensor_copy(oth[u][qb][:, i, :], nf[:])
                        eng.tensor_tensor(otl[u][qb][:, i, :], nf[:],
                                          oth[u][qb][:, i, :], sub)
                    return tail

                def mk_cchunk(qb, ti, nn, ysb, last):
                    def cchunk(u=None):
                        # u=None: both head-pairs in one psum group.
                        # u=0/1: split passes (last qb) -- pair-0 matmuls can
                        # run while pair-1's softmax tail is still finishing.
                        ts_ = slice(ti * 128, (ti + 1) * 128)
                        ns_ = slice(nn * 512, (nn + 1) * 512)
                        ys = ysb[:, nn * 512:(nn + 1) * 512]
                        us = (0, 1) if u is None else (u,)
                        psy = cps.tile([128, 512], f32, tag="psy", name="psy")
                        for uu in us:
                            for m, (a, w) in enumerate(
                                    ((oth, woh), (oth, wol), (otl, woh))):
                                nc.tensor.matmul(
                                    psy[:], a[uu][qb][:, :, ts_], w[uu][:, :, ns_],
                                    perf_mode=DR,
                                    start=(uu == us[0] and m == 0),
                                    stop=(uu == us[-1] and m == 2))
                        if u == 0:
                            nc.vector.tensor_copy(ys, psy[:])
                            return
                        if u == 1:
                            nc.vector.tensor_tensor(ys, ys, psy[:], add)
                        elif last:
                            nc.scalar.activation(ys, psy[:], Copy)
                        else:
                            nc.vector.tensor_copy(ys, psy[:])
                        qtile = qb * 4 + ti
                        if last:
                            # final tile: store per-slice so the tail DMA is
                            # small and the drain starts sooner
                            nc.sync.dma_start(
                                y_d[qtile * 128:(qtile + 1) * 128,
                                    nn * 512:(nn + 1) * 512], ys)
                        elif nn == NCH - 1:
                            nc.sync.dma_start(
                                y_d[qtile * 128:(qtile + 1) * 128, :], ysb[:])
                    return cchunk

                for qb in range(NCH):
                    for h in range(NH):
                        pso = bps.tile([128, 512], f32, tag=f"pso{(qb * 4 + h) % 2}",
                                       name="pso")
                        saccs, es_tiles = [], []

                        def consume(pt):
                            # AV + level-1 row-sum for es_tiles[pt], one step
                            # behind the S/exp stream so PE never waits on Act
                            es = es_tiles[pt]
                            nc.tensor.matmul(pso[:], vn[2 * pt][:], es[:, 0, :],
                                             start=(pt == 0), stop=False)
                            nc.tensor.matmul(pso[:], vn[2 * pt + 1][:],
                                             es[:, 1, :],
                                             start=False, stop=(pt == 7))
                            if pt % 2 == 1:
                                sa = bsa.tile([128, 2, 512], fp16,
                                              tag=f"sa{pt // 2}",
                                              name=f"sa{pt // 2}")
                                nc.vector.tensor_tensor(sa[:], es_tiles[pt - 1][:],
                                                        es[:], add)
                                saccs.append(sa)

                        for pt in range(8):
                            pss = bps.tile([128, 2, 512], f32,
                                           tag=f"pss{pt % 2}", name=f"pss{pt % 2}")
                            for half in range(2):
                                k = 2 * pt + half
                                nc.tensor.matmul(
                                    pss[:, half, :],
                                    kt[k // 4][:, (k % 4) * 128:(k % 4 + 1) * 128],
                                    qt[h][qb][:], start=True, stop=True)
                            es = bx.tile([128, 2, 512], fp16, tag="es", name="es")
                            nc.scalar.activation(es[:], pss[:], Exp, scale=SCALE)
                            es_tiles.append(es)
                            if pt > 0:
                                consume(pt - 1)
                            pump(1)
                        consume(7)
                        workq.append(mk_tail(h, qb, saccs, pso))
                    # let the last head's tail chain (~5us of DVE/Pool
                    # latency) finish before the first C chunk needs its ot
                    workq.extend([lambda: None] * 4)
                    for ti in range(4):
                        ysb = yp.tile([128, L], f32, tag="ysb", name="ysb")
                        for nn in range(NCH):
                            workq.append(mk_cchunk(qb, ti, nn, ysb,
                                                   qb == NCH - 1))
                pump(len(workq))

    nc.compile()
    return nc


def _host_inputs(x, Wq, Wk, Wv, Wo):
    inv = 1.0 / (BASE ** (np.arange(0, HD, 2, dtype=np.float32) / HD))
    pos = np.arange(L, dtype=np.float32)
    fr = pos[:, None] * inv[None, :]
    emb = np.concatenate([fr, fr], axis=1)            # [L, HD]
    cosT = np.ascontiguousarray(np.cos(emb).T).astype(BF16)
    sinT = np.ascontiguousarray(np.sin(emb).T).astype(BF16)
    shp = np.zeros((HD, HD), np.float32)
    shp[(np.arange(HD) - 1) % HD, np.arange(HD)] = 1.0
    idn = np.eye(128, dtype=np.float32)

    FP8 = ml_dtypes.float8_e4m3
    FP8E5 = ml_dtypes.float8_e5m2

    def hilo(a):
        hi = a.astype(FP8)
        lo = (a - hi.astype(np.float32)).astype(FP8E5)
        return hi, lo

    maps = []
    for c in range(8):
        b, g = c // 4, c % 4
        xT = x[b].T                                    # [D, L]
        # [p, n, t, i, m] = xT[256t+128i+p, 512n+m]
        xa = xT.reshape(8, 2, 128, NCH, 512).transpose(2, 3, 0, 1, 4)
        x8h, x8l = hilo(np.ascontiguousarray(xa))
        # weights: [p, t, i, m] = 32*W[256t+128i+p, m]
        wq = (Wq[:, g * NH * HD:(g + 1) * NH * HD] * WS)
        wq8h, wq8l = hilo(np.ascontiguousarray(
            wq.reshape(8, 2, 128, NH * HD).transpose(2, 0, 1, 3)))
        wk = (Wk[:, g * HD:(g + 1) * HD] * WS)
        wk8h, wk8l = hilo(np.ascontiguousarray(
            wk.reshape(8, 2, 128, HD).transpose(2, 0, 1, 3)))
        wv = (Wv[:, g * HD:(g + 1) * HD] * WS)
        wv8h, wv8l = hilo(np.ascontiguousarray(
            wv.reshape(8, 2, 128, HD).transpose(2, 0, 1, 3)))
        # wo pairs: [u, p, i, m] = 32*Wo[g*512 + (2u+i)*128 + p, m]
        wo = (Wo[g * NH * HD:(g + 1) * NH * HD, :] * WS)
        wo8h, wo8l = hilo(np.ascontiguousarray(
            wo.reshape(2, 2, 128, D).transpose(0, 2, 1, 3)))
        maps.append({
            "x8h": x8h, "x8l": x8l,
            "wq8h": wq8h, "wq8l": wq8l, "wk8h": wk8h, "wk8l": wk8l,
            "wv8h": wv8h, "wv8l": wv8l, "wo8h": wo8h, "wo8l": wo8l,
            "cosT": cosT, "sinT": sinT,
            "shiftP": shp.astype(BF16), "ident": idn.astype(BF16),
        })
    return maps


def _run(inputs, trace=False):
    global _compiled
    from concourse.bass_utils import run_bass_kernel_spmd
    if _compiled is None:
        _compiled = _build()
    maps = _host_inputs(inputs["x"], inputs["Wq"], inputs["Wk"],
                        inputs["Wv"], inputs["Wo"])
    res = run_bass_kernel_spmd(_compiled, maps, list(range(8)), trace=trace)
    y = np.empty((B, L, D), np.float32)
    for b in range(B):
        y[b] = res.results[b * 4]["y"]
        for g in range(1, 4):
            y[b] += res.results[b * 4 + g]["y"]
    y *= 1.0 / (WS * WS)   # v and wo each carry the x32 host prescale
    return y, res


def kernel(**inputs):
    x = np.asarray(inputs["x"], np.float32)
    y, _ = _run({"x": x,
                 "Wq": np.asarray(inputs["Wq"], np.float32),
                 "Wk": np.asarray(inputs["Wk"], np.float32),
                 "Wv": np.asarray(inputs["Wv"], np.float32),
                 "Wo": np.asarray(inputs["Wo"], np.float32)})
    return y


# revision 69
# speedup vs baseline: 1.0197x; 1.0176x over previous
"""GQA attention kernel for 8 trn2 NeuronCores.

Sharding: core c in 0..7 -> batch b = c//4, KV group g = c%4 (4 Q heads,
1 KV head per core). Tensor-parallel on Wq/Wk/Wv columns and Wo rows;
host sums the 4 partial outputs per batch.

Precision: softmax-weight noise passes 1:1 to the output (the output is a
weighted mean, so its scale shrinks with the same sqrt(N) that averages the
noise). fp8 anywhere on the Q/K/exp path therefore fails the 2e-2 gate
(measured ~3e-2 per stage); the whole attention core runs bf16/fp16, which
lands ~3e-3. All matmuls run at 1 cycle/row (full PE rate).

Speed comes from engine balance and occupancy:
- softmax row-sums: DVE pairwise tree (fp16 2x/4x modes) + Pool level-2,
  one final ones-matmul pair on PE instead of 8 M=1 matmuls;
- PSUM->SBUF copies on Act (phase A) / DVE (phase C); Pool does the
  SBUF-side RoPE multiplies (it cannot touch PSUM);
- softmax tails and out-projection chunks are deferred and woven into the
  next head's S/exp stream so in-order PE never stalls on DVE latency.
"""
import sys
sys.path.insert(0, "/opt/trn_rl_repo")
import math
import numpy as np
import ml_dtypes

B, L, D = 2, 2048, 2048
H, HKV, HD = 16, 4, 128
BASE = 10000.0
NCH = L // 512     # 4 seq chunks of 512
NH = H // HKV      # 4 heads per core
WS = 32.0          # host prescale on W (keeps fp8-hi in the normal range);
                   # q,k carry x32 -> exp scale divides by 32*32; v carries
                   # x32 -> host divides y by 32
SCALE = 1.0 / (math.sqrt(HD) * WS * WS)

FP16 = np.float16
BF16 = ml_dtypes.bfloat16

_compiled = None


def _build():
    from concourse import bacc, tile, mybir

    f32, f32r = mybir.dt.float32, mybir.dt.float32r
    bf16, fp16 = mybir.dt.bfloat16, mybir.dt.float16
    Exp = mybir.ActivationFunctionType.Exp
    Copy = mybir.ActivationFunctionType.Copy
    mult, add, sub = (mybir.AluOpType.mult, mybir.AluOpType.add,
                      mybir.AluOpType.subtract)

    nc = bacc.Bacc("TRN2", target_bir_lowering=False, debug=False,
                   enable_asserts=True, num_devices=8)

    fp8, fp8e5 = mybir.dt.float8e4, mybir.dt.float8e5
    DR = mybir.MatmulPerfMode.DoubleRow
    x8h_d = nc.dram_tensor("x8h", [128, NCH, 8, 2, 512], fp8, kind="ExternalInput")
    x8l_d = nc.dram_tensor("x8l", [128, NCH, 8, 2, 512], fp8e5,
                           kind="ExternalInput")
    wq8h_d = nc.dram_tensor("wq8h", [128, 8, 2, 512], fp8, kind="ExternalInput")
    wq8l_d = nc.dram_tensor("wq8l", [128, 8, 2, 512], fp8e5, kind="ExternalInput")
    wk8h_d = nc.dram_tensor("wk8h", [128, 8, 2, 128], fp8, kind="ExternalInput")
    wk8l_d = nc.dram_tensor("wk8l", [128, 8, 2, 128], fp8e5, kind="ExternalInput")
    wv8h_d = nc.dram_tensor("wv8h", [128, 8, 2, 128], fp8, kind="ExternalInput")
    wv8l_d = nc.dram_tensor("wv8l", [128, 8, 2, 128], fp8e5, kind="ExternalInput")
    wo8h_d = nc.dram_tensor("wo8h", [2, 128, 2, D], fp8, kind="ExternalInput")
    wo8l_d = nc.dram_tensor("wo8l", [2, 128, 2, D], fp8e5, kind="ExternalInput")
    cos_d = nc.dram_tensor("cosT", [HD, L], bf16, kind="ExternalInput")
    sin_d = nc.dram_tensor("sinT", [HD, L], bf16, kind="ExternalInput")
    shp_d = nc.dram_tensor("shiftP", [HD, HD], bf16, kind="ExternalInput")
    idn_d = nc.dram_tensor("ident", [128, 128], bf16, kind="ExternalInput")
    y_d = nc.dram_tensor("y", [L, D], f32, kind="ExternalOutput")

    with tile.TileContext(nc) as tc, \
         nc.allow_low_precision(reason="bf16/fp16 attention core; see module "
                                "docstring noise analysis"):
        with tc.tile_pool(name="persist", bufs=1) as pp:
            qt = [[pp.tile([HD, 512], bf16, tag=f"qt{h}_{n}", name=f"qt{h}_{n}")
                   for n in range(NCH)] for h in range(NH)]
            kt = [pp.tile([HD, 512], bf16, tag=f"kt{n}", name=f"kt{n}")
                  for n in range(NCH)]
            vn = [pp.tile([128, HD], fp16, tag=f"vn{t}", name=f"vn{t}")
                  for t in range(16)]
            # attention outputs in hi/lo fp8, head-PAIRED on dim 1 for the
            # DoubleRow out-projection
            oth = [[pp.tile([HD, 2, 512], fp8, tag=f"oth{u}_{n}",
                            name=f"oth{u}_{n}") for n in range(NCH)]
                   for u in range(2)]
            otl = [[pp.tile([HD, 2, 512], fp8e5, tag=f"otl{u}_{n}",
                            name=f"otl{u}_{n}") for n in range(NCH)]
                   for u in range(2)]
            woh = [pp.tile([HD, 2, L], fp8, tag=f"woh{u}", name=f"woh{u}")
                   for u in range(2)]
            wol = [pp.tile([HD, 2, L], fp8e5, tag=f"wol{u}", name=f"wol{u}")
                   for u in range(2)]
            shp = pp.tile([HD, HD], bf16, tag="shp", name="shp")
            idn = pp.tile([128, 128], bf16, tag="idn", name="idn")
            cosT = pp.tile([HD, L], bf16, tag="cos", name="cos")
            sinT = pp.tile([HD, L], bf16, tag="sin", name="sin")

            # ---------------- Phase A: projections + RoPE + V transpose ------
            with tc.tile_pool(name="aw", bufs=1) as aw, \
                 tc.tile_pool(name="ax", bufs=1) as ax, \
                 tc.tile_pool(name="atmp", bufs=3) as at, \
                 tc.tile_pool(name="apsum", bufs=1, space="PSUM") as aps:
                wqh = aw.tile([128, 8, 2, 512], fp8, tag="wqh", name="wqh")
                wql = aw.tile([128, 8, 2, 512], fp8e5, tag="wql", name="wql")
                wkh = aw.tile([128, 8, 2, 128], fp8, tag="wkh", name="wkh")
                wkl = aw.tile([128, 8, 2, 128], fp8e5, tag="wkl", name="wkl")
                wvh = aw.tile([128, 8, 2, 128], fp8, tag="wvh", name="wvh")
                wvl = aw.tile([128, 8, 2, 128], fp8e5, tag="wvl", name="wvl")
                xhs = [ax.tile([128, 8, 2, 512], fp8, tag=f"xh{i}", name=f"xh{i}")
                       for i in range(2)]
                xls = [ax.tile([128, 8, 2, 512], fp8e5, tag=f"xl{i}",
                               name=f"xl{i}") for i in range(2)]
                # DMA issue order: what the first matmuls need, first.
                # (single SP queue executes in order; wo waits until phase B)
                for sl in (slice(0, 2), slice(2, 4), slice(4, 6),
                           slice(6, 8)):
                    nc.sync.dma_start(wqh[:, sl], wq8h_d[:, sl])
                    nc.sync.dma_start(xhs[0][:, sl], x8h_d[:, 0, sl])
                    nc.sync.dma_start(xls[0][:, sl], x8l_d[:, 0, sl])
                    nc.sync.dma_start(wql[:, sl], wq8l_d[:, sl])
                    nc.sync.dma_start(wkh[:, sl], wk8h_d[:, sl])
                    nc.sync.dma_start(wkl[:, sl], wk8l_d[:, sl])
                    nc.sync.dma_start(wvh[:, sl], wv8h_d[:, sl])
                    nc.sync.dma_start(wvl[:, sl], wv8l_d[:, sl])
                nc.sync.dma_start(idn[:], idn_d[:])
                for n in range(NCH):
                    ps = [aps.tile([128, 512], f32, tag=f"pa{j}", name=f"pa{j}")
                          for j in range(6)]
                    xh, xl = xhs[n % 2], xls[n % 2]
                    if n + 1 < NCH:   # prefetch next chunk
                        for q in range(4):
                            sl = slice(2 * q, 2 * (q + 1))
                            nc.sync.dma_start(xhs[(n + 1) % 2][:, sl],
                                              x8h_d[:, n + 1, sl])
                            nc.sync.dma_start(xls[(n + 1) % 2][:, sl],
                                              x8l_d[:, n + 1, sl])
                    if n == 0:
                        # needed only from the chunk-0 RoPE onward; issued
                        # after the chunk-1 prefetch so that isn't delayed
                        nc.sync.dma_start(cosT[:], cos_d[:])
                        nc.sync.dma_start(sinT[:], sin_d[:])
                        nc.sync.dma_start(shp[:], shp_d[:])
                        # preload the Exp table while Act is idle so the
                        # first phase-B exp doesn't pay LoadActFuncSet
                        warm = at.tile([1, 8], f32, tag="warm", name="warm")
                        nc.scalar.activation(warm[:], shp[0:1, 0:8], Exp)
                    # hi/lo fp8 DoubleRow: exact - (x_lo @ w_lo); contraction
                    # pairs c-tiles (2t, 2t+1) on dim 1. At t=7, v/k groups
                    # stop first so their PSUM->SBUF copies start ASAP.
                    for t in range(8):
                        kv = ((5, wvh, wvl), (4, wkh, wkl))
                        for pj, wh_, wl_ in (kv if t == 7 else ()):
                            for wt, xt in ((wh_, xh), (wh_, xl), (wl_, xh)):
                                nc.tensor.matmul(
                                    ps[pj][:], wt[:, t], xt[:, t],
                                    perf_mode=DR, start=False,
                                    stop=(wt is wl_))
                        for j in range(NH):
                            js = slice(j * 128, (j + 1) * 128)
                            for wt, xt in ((wqh, xh), (wqh, xl), (wql, xh)):
                                nc.tensor.matmul(
                                    ps[j][:], wt[:, t, :, js], xt[:, t],
                                    perf_mode=DR,
                                    start=(t == 0 and xt is xh and wt is wqh),
                                    stop=(t == 7 and wt is wql))
                        if t < 7:
                            for pj, wh_, wl_ in kv:
                                for wt, xt in ((wh_, xh), (wh_, xl), (wl_, xh)):
                                    nc.tensor.matmul(
                                        ps[pj][:], wt[:, t], xt[:, t],
                                        perf_mode=DR,
                                        start=(t == 0 and xt is xh
                                               and wt is wh_),
                                        stop=False)
                    cs = cosT[:, n * 512:(n + 1) * 512]
                    sn = sinT[:, n * 512:(n + 1) * 512]
                    # vf first: PE's next work (transposes) depends on it;
                    # k's RoPE first: phase B's S matmuls depend on kt
                    vf = at.tile([128, 512], bf16, tag="vf", name="vf")
                    nc.scalar.activation(vf[:], ps[5][:], Copy)
                    raws = {}
                    for j in (4, 0, 1, 2, 3):
                        raw = at.tile([128, 512], bf16, tag=f"raw{j}",
                                      name=f"raw{j}", bufs=2)
                        nc.scalar.activation(raw[:], ps[j][:], Copy)
                        # roll along HD via partition-shifted SBUF copies
                        rol = at.tile([128, 512], bf16, tag=f"rol{j}",
                                      name=f"rol{j}", bufs=2)
                        nc.sync.dma_start(rol[1:128, :], raw[0:127, :])
                        nc.sync.dma_start(rol[0:1, :], raw[127:128, :])
                        raws[j] = (raw, rol)
                        if j == 4:
                            for t in range(4):
                                pvt = aps.tile([128, 128], bf16, tag="pvt",
                                               name="pvt")
                                nc.tensor.transpose(
                                    pvt[:], vf[:, t * 128:(t + 1) * 128], idn[:])
                                nc.vector.tensor_copy(vn[n * 4 + t][:], pvt[:])
                    for j in (4, 0, 1, 2, 3):
                        raw, rol = raws[j]
                        t1 = at.tile([128, 512], bf16, tag="t1", name="t1")
                        nc.gpsimd.tensor_tensor(t1[:], raw[:], cs, mult)
                        t2 = at.tile([128, 512], bf16, tag="t2", name="t2")
                        nc.vector.tensor_tensor(t2[:], rol[:], sn, mult)
                        dst = qt[j][n] if j < NH else kt[n]
                        nc.vector.tensor_tensor(dst[:], t1[:], t2[:], add)

            # ------------- Phase B: attention; Phase C: out-projection -------
            # Deferred-work queue: softmax tails and out-projection chunks are
            # emitted interleaved with later heads' S/exp stream so the
            # in-order PE queue never waits on the DVE/Pool reduction chain.
            with tc.tile_pool(name="bexp", bufs=6) as bx, \
                 tc.tile_pool(name="bsacc", bufs=2) as bsa, \
                 tc.tile_pool(name="bsm", bufs=2) as bs, \
                 tc.tile_pool(name="yout", bufs=4) as yp, \
                 tc.tile_pool(name="bpsum", bufs=1, space="PSUM") as bps, \
                 tc.tile_pool(name="cpsum", bufs=2, space="PSUM") as cps:
                workq = []

                def pump(k):
                    for _ in range(min(k, len(workq))):
                        workq.pop(0)()

                from concourse import bass_isa
                for u in range(2):
                    nc.sync.dma_start(woh[u][:], wo8h_d[u])
                    nc.sync.dma_start(wol[u][:], wo8l_d[u])

                def mk_tail(h, qb, saccs, pso):
                    def tail():
                        # all-DVE tree to one [128,512] tile, then a gpsimd
                        # partition all-reduce gives every partition the row
                        # sum -- no PSUM, no ones-matmul, no broadcast matmul
                        l2a = bsa.tile([128, 2, 512], fp16, tag="l2a", name="l2a")
                        nc.vector.tensor_tensor(l2a[:], saccs[0][:], saccs[1][:],
                                                add)
                        l2b = bsa.tile([128, 2, 512], fp16, tag="l2b", name="l2b")
                        nc.vector.tensor_tensor(l2b[:], saccs[2][:], saccs[3][:],
                                                add)
                        l3 = bsa.tile([128, 2, 512], fp16, tag="l3", name="l3")
                        nc.vector.tensor_tensor(l3[:], l2a[:], l2b[:], add)
                        sht = bsa.tile([128, 512], f32, tag="sht", name="sht")
                        nc.vector.tensor_tensor(sht[:], l3[:, 0, :], l3[:, 1, :],
                                                add)
                        sums = bsa.tile([128, 512], f32, tag="sums", name="sums")
                        nc.gpsimd.partition_all_reduce(sums[:], sht[:], 128,
                                                       bass_isa.ReduceOp.add)
                        rec = bs.tile([128, 512], f32r, tag="rec", name="rec")
                        nc.vector.reciprocal(rec[:], sums[:])
                        nf = bs.tile([128, 512], f32, tag="nf", name="nf")
                        nc.vector.tensor_tensor(nf[:], pso[:], rec[:], mult)
                        u, i = h // 2, h % 2
                        # last qb: DVE for the hi/lo split -- it is on the
                        # critical chain into the final out-projection drain
                        eng = nc.vector if qb == NCH - 1 else nc.gpsimd
                        eng.tensor_copy(oth[u][qb][:, i, :], nf[:])
                        eng.tensor_tensor(otl[u][qb][:, i, :], nf[:],
                                          oth[u][qb][:, i, :], sub)
                    return tail

                def mk_cchunk(qb, ti, nn, ysb, last):
                    def cchunk(u=None):
                        # u=None: both head-pairs in one psum group.
                        # u=0/1: split passes (last qb) -- pair-0 matmuls can
                        # run while pair-1's softmax tail is still finishing.
                        ts_ = slice(ti * 128, (ti + 1) * 128)
                        ns_ = slice(nn * 512, (nn + 1) * 512)
                        ys = ysb[:, nn * 512:(nn + 1) * 512]
                        us = (0, 1) if u is None else (u,)
                        psy = cps.tile([128, 512], f32, tag="psy", name="psy")
                        for uu in us:
                            for m, (a, w) in enumerate(
                                    ((oth, woh), (oth, wol), (otl, woh))):
                                nc.tensor.matmul(
                                    psy[:], a[uu][qb][:, :, ts_], w[uu][:, :, ns_],
                                    perf_mode=DR,
                                    start=(uu == us[0] and m == 0),
                                    stop=(uu == us[-1] and m == 2))
                        if u == 0:
                            nc.vector.tensor_copy(ys, psy[:])
                            return
                        if u == 1:
                            nc.vector.tensor_tensor(ys, ys, psy[:], add)
                        elif last:
                            nc.scalar.activation(ys, psy[:], Copy)
                        else:
                            nc.vector.tensor_copy(ys, psy[:])
                        qtile = qb * 4 + ti
                        if last:
                            # final tile: store per-slice so the tail DMA is
                            # small and the drain starts sooner
                            nc.sync.dma_start(
                                y_d[qtile * 128:(qtile + 1) * 128,
                                    nn * 512:(nn + 1) * 512], ys)
                        elif nn == NCH - 1:
                            nc.sync.dma_start(
                                y_d[qtile * 128:(qtile + 1) * 128, :], ysb[:])
                    return cchunk

                for qb in range(NCH):
                    for h in range(NH):
                        pso = bps.tile([128, 512], f32, tag=f"pso{(qb * 4 + h) % 2}",
                                       name="pso")
                        saccs, es_tiles = [], []

                        def consume(pt):
                            # AV + level-1 row-sum for es_tiles[pt], one step
                            # behind the S/exp stream so PE never waits on Act
                            es = es_tiles[pt]
                            nc.tensor.matmul(pso[:], vn[2 * pt][:], es[:, 0, :],
                                             start=(pt == 0), stop=False)
                            nc.tensor.matmul(pso[:], vn[2 * pt + 1][:],
                                             es[:, 1, :],
                                             start=False, stop=(pt == 7))
                            if pt % 2 == 1:
                                sa = bsa.tile([128, 2, 512], fp16,
                                              tag=f"sa{pt // 2}",
                                              name=f"sa{pt // 2}")
                                nc.vector.tensor_tensor(sa[:], es_tiles[pt - 1][:],
                                                        es[:], add)
                                saccs.append(sa)

                        for pt in range(8):
                            pss = bps.tile([128, 2, 512], f32,
                                           tag=f"pss{pt % 2}", name=f"pss{pt % 2}")
                            for half in range(2):
                                k = 2 * pt + half
                                nc.tensor.matmul(
                                    pss[:, half, :],
                                    kt[k // 4][:, (k % 4) * 128:(k % 4 + 1) * 128],
                                    qt[h][qb][:], start=True, stop=True)
                            es = bx.tile([128, 2, 512], fp16, tag="es", name="es")
                            nc.scalar.activation(es[:], pss[:], Exp, scale=SCALE)
                            es_tiles.append(es)
                            if pt > 0:
                                consume(pt - 1)
                            pump(1)
                        consume(7)
                        workq.append(mk_tail(h, qb, saccs, pso))
                    # let the last head's tail chain (~5us of DVE/Pool
                    # latency) finish before the first C chunk needs its ot
                    workq.extend([lambda: None] * 6)
                    for ti in range(4):
                        ysb = yp.tile([128, L], f32, tag="ysb", name="ysb")
                        for nn in range(NCH):
                            workq.append(mk_cchunk(qb, ti, nn, ysb,
                                                   qb == NCH - 1))
                pump(len(workq))

    nc.compile()
    return nc


def _host_inputs(x, Wq, Wk, Wv, Wo):
    inv = 1.0 / (BASE ** (np.arange(0, HD, 2, dtype=np.float32) / HD))
    pos = np.arange(L, dtype=np.float32)
    fr = pos[:, None] * inv[None, :]
    emb = np.concatenate([fr, fr], axis=1)            # [L, HD]
    cosT = np.ascontiguousarray(np.cos(emb).T).astype(BF16)
    sinT = np.ascontiguousarray(np.sin(emb).T).astype(BF16)
    shp = np.zeros((HD, HD), np.float32)
    shp[(np.arange(HD) - 1) % HD, np.arange(HD)] = 1.0
    idn = np.eye(128, dtype=np.float32)

    FP8 = ml_dtypes.float8_e4m3
    FP8E5 = ml_dtypes.float8_e5m2

    def hilo(a):
        hi = a.astype(FP8)
        lo = (a - hi.astype(np.float32)).astype(FP8E5)
        return hi, lo

    maps = []
    for c in range(8):
        b, g = c // 4, c % 4
        xT = x[b].T                                    # [D, L]
        # [p, n, t, i, m] = xT[256t+128i+p, 512n+m]
        xa = xT.reshape(8, 2, 128, NCH, 512).transpose(2, 3, 0, 1, 4)
        x8h, x8l = hilo(np.ascontiguousarray(xa))
        # weights: [p, t, i, m] = 32*W[256t+128i+p, m]
        wq = (Wq[:, g * NH * HD:(g + 1) * NH * HD] * WS)
        wq8h, wq8l = hilo(np.ascontiguousarray(
            wq.reshape(8, 2, 128, NH * HD).transpose(2, 0, 1, 3)))
        wk = (Wk[:, g * HD:(g + 1) * HD] * WS)
        wk8h, wk8l = hilo(np.ascontiguousarray(
            wk.reshape(8, 2, 128, HD).transpose(2, 0, 1, 3)))
        wv = (Wv[:, g * HD:(g + 1) * HD] * WS)
        wv8h, wv8l = hilo(np.ascontiguousarray(
            wv.reshape(8, 2, 128, HD).transpose(2, 0, 1, 3)))
        # wo pairs: [u, p, i, m] = 32*Wo[g*512 + (2u+i)*128 + p, m]
        wo = (Wo[g * NH * HD:(g + 1) * NH * HD, :] * WS)
        wo8h, wo8l = hilo(np.ascontiguousarray(
            wo.reshape(2, 2, 128, D).transpose(0, 2, 1, 3)))
        maps.append({
            "x8h": x8h, "x8l": x8l,
            "wq8h": wq8h, "wq8l": wq8l, "wk8h": wk8h, "wk8l": wk8l,
            "wv8h": wv8h, "wv8l": wv8l, "wo8h": wo8h, "wo8l": wo8l,
            "cosT": cosT, "sinT": sinT,
            "shiftP": shp.astype(BF16), "ident": idn.astype(BF16),
        })
    return maps


def _run(inputs, trace=False):
    global _compiled
    from concourse.bass_utils import run_bass_kernel_spmd
    if _compiled is None:
        _compiled = _build()
    maps = _host_inputs(inputs["x"], inputs["Wq"], inputs["Wk"],
                        inputs["Wv"], inputs["Wo"])
    res = run_bass_kernel_spmd(_compiled, maps, list(range(8)), trace=trace)
    y = np.empty((B, L, D), np.float32)
    for b in range(B):
        y[b] = res.results[b * 4]["y"]
        for g in range(1, 4):
            y[b] += res.results[b * 4 + g]["y"]
    y *= 1.0 / (WS * WS)   # v and wo each carry the x32 host prescale
    return y, res


def kernel(**inputs):
    x = np.asarray(inputs["x"], np.float32)
    y, _ = _run({"x": x,
                 "Wq": np.asarray(inputs["Wq"], np.float32),
                 "Wk": np.asarray(inputs["Wk"], np.float32),
                 "Wv": np.asarray(inputs["Wv"], np.float32),
                 "Wo": np.asarray(inputs["Wo"], np.float32)})
    return y


# revision 70
# speedup vs baseline: 1.0231x; 1.0034x over previous
"""GQA attention kernel for 8 trn2 NeuronCores.

Sharding: core c in 0..7 -> batch b = c//4, KV group g = c%4 (4 Q heads,
1 KV head per core). Tensor-parallel on Wq/Wk/Wv columns and Wo rows;
host sums the 4 partial outputs per batch.

Precision: softmax-weight noise passes 1:1 to the output (the output is a
weighted mean, so its scale shrinks with the same sqrt(N) that averages the
noise). fp8 anywhere on the Q/K/exp path therefore fails the 2e-2 gate
(measured ~3e-2 per stage); the whole attention core runs bf16/fp16, which
lands ~3e-3. All matmuls run at 1 cycle/row (full PE rate).

Speed comes from engine balance and occupancy:
- softmax row-sums: DVE pairwise tree (fp16 2x/4x modes) + Pool level-2,
  one final ones-matmul pair on PE instead of 8 M=1 matmuls;
- PSUM->SBUF copies on Act (phase A) / DVE (phase C); Pool does the
  SBUF-side RoPE multiplies (it cannot touch PSUM);
- softmax tails and out-projection chunks are deferred and woven into the
  next head's S/exp stream so in-order PE never stalls on DVE latency.
"""
import sys
sys.path.insert(0, "/opt/trn_rl_repo")
import math
import numpy as np
import ml_dtypes

B, L, D = 2, 2048, 2048
H, HKV, HD = 16, 4, 128
BASE = 10000.0
NCH = L // 512     # 4 seq chunks of 512
NH = H // HKV      # 4 heads per core
WS = 32.0          # host prescale on W (keeps fp8-hi in the normal range);
                   # q,k carry x32 -> exp scale divides by 32*32; v carries
                   # x32 -> host divides y by 32
SCALE = 1.0 / (math.sqrt(HD) * WS * WS)

FP16 = np.float16
BF16 = ml_dtypes.bfloat16

_compiled = None


def _build():
    from concourse import bacc, tile, mybir

    f32, f32r = mybir.dt.float32, mybir.dt.float32r
    bf16, fp16 = mybir.dt.bfloat16, mybir.dt.float16
    Exp = mybir.ActivationFunctionType.Exp
    Copy = mybir.ActivationFunctionType.Copy
    mult, add, sub = (mybir.AluOpType.mult, mybir.AluOpType.add,
                      mybir.AluOpType.subtract)

    nc = bacc.Bacc("TRN2", target_bir_lowering=False, debug=False,
                   enable_asserts=True, num_devices=8)

    fp8, fp8e5 = mybir.dt.float8e4, mybir.dt.float8e5
    DR = mybir.MatmulPerfMode.DoubleRow
    x8h_d = nc.dram_tensor("x8h", [128, NCH, 8, 2, 512], fp8, kind="ExternalInput")
    x8l_d = nc.dram_tensor("x8l", [128, NCH, 8, 2, 512], fp8e5,
                           kind="ExternalInput")
    wq8h_d = nc.dram_tensor("wq8h", [128, 8, 2, 512], fp8, kind="ExternalInput")
    wq8l_d = nc.dram_tensor("wq8l", [128, 8, 2, 512], fp8e5, kind="ExternalInput")
    wk8h_d = nc.dram_tensor("wk8h", [128, 8, 2, 128], fp8, kind="ExternalInput")
    wk8l_d = nc.dram_tensor("wk8l", [128, 8, 2, 128], fp8e5, kind="ExternalInput")
    wv8h_d = nc.dram_tensor("wv8h", [128, 8, 2, 128], fp8, kind="ExternalInput")
    wv8l_d = nc.dram_tensor("wv8l", [128, 8, 2, 128], fp8e5, kind="ExternalInput")
    wo8h_d = nc.dram_tensor("wo8h", [2, 128, 2, D], fp8, kind="ExternalInput")
    wo8l_d = nc.dram_tensor("wo8l", [2, 128, 2, D], fp8e5, kind="ExternalInput")
    cos_d = nc.dram_tensor("cosT", [HD, L], bf16, kind="ExternalInput")
    sin_d = nc.dram_tensor("sinT", [HD, L], bf16, kind="ExternalInput")
    shp_d = nc.dram_tensor("shiftP", [HD, HD], bf16, kind="ExternalInput")
    idn_d = nc.dram_tensor("ident", [128, 128], bf16, kind="ExternalInput")
    y_d = nc.dram_tensor("y", [L, D], f32, kind="ExternalOutput")

    with tile.TileContext(nc) as tc, \
         nc.allow_low_precision(reason="bf16/fp16 attention core; see module "
                                "docstring noise analysis"):
        with tc.tile_pool(name="persist", bufs=1) as pp:
            qt = [[pp.tile([HD, 512], bf16, tag=f"qt{h}_{n}", name=f"qt{h}_{n}")
                   for n in range(NCH)] for h in range(NH)]
            kt = [pp.tile([HD, 512], bf16, tag=f"kt{n}", name=f"kt{n}")
                  for n in range(NCH)]
            vn = [pp.tile([128, HD], fp16, tag=f"vn{t}", name=f"vn{t}")
                  for t in range(16)]
            # attention outputs in hi/lo fp8, head-PAIRED on dim 1 for the
            # DoubleRow out-projection
            oth = [[pp.tile([HD, 2, 512], fp8, tag=f"oth{u}_{n}",
                            name=f"oth{u}_{n}") for n in range(NCH)]
                   for u in range(2)]
            otl = [[pp.tile([HD, 2, 512], fp8e5, tag=f"otl{u}_{n}",
                            name=f"otl{u}_{n}") for n in range(NCH)]
                   for u in range(2)]
            woh = [pp.tile([HD, 2, L], fp8, tag=f"woh{u}", name=f"woh{u}")
                   for u in range(2)]
            wol = [pp.tile([HD, 2, L], fp8e5, tag=f"wol{u}", name=f"wol{u}")
                   for u in range(2)]
            shp = pp.tile([HD, HD], bf16, tag="shp", name="shp")
            idn = pp.tile([128, 128], bf16, tag="idn", name="idn")
            cosT = pp.tile([HD, L], bf16, tag="cos", name="cos")
            sinT = pp.tile([HD, L], bf16, tag="sin", name="sin")

            # ---------------- Phase A: projections + RoPE + V transpose ------
            with tc.tile_pool(name="aw", bufs=1) as aw, \
                 tc.tile_pool(name="ax", bufs=1) as ax, \
                 tc.tile_pool(name="atmp", bufs=3) as at, \
                 tc.tile_pool(name="apsum", bufs=1, space="PSUM") as aps:
                wqh = aw.tile([128, 8, 2, 512], fp8, tag="wqh", name="wqh")
                wql = aw.tile([128, 8, 2, 512], fp8e5, tag="wql", name="wql")
                wkh = aw.tile([128, 8, 2, 128], fp8, tag="wkh", name="wkh")
                wkl = aw.tile([128, 8, 2, 128], fp8e5, tag="wkl", name="wkl")
                wvh = aw.tile([128, 8, 2, 128], fp8, tag="wvh", name="wvh")
                wvl = aw.tile([128, 8, 2, 128], fp8e5, tag="wvl", name="wvl")
                xhs = [ax.tile([128, 8, 2, 512], fp8, tag=f"xh{i}", name=f"xh{i}")
                       for i in range(2)]
                xls = [ax.tile([128, 8, 2, 512], fp8e5, tag=f"xl{i}",
                               name=f"xl{i}") for i in range(2)]
                # DMA issue order: what the first matmuls need, first.
                # (single SP queue executes in order; wo waits until phase B)
                for sl in (slice(0, 2), slice(2, 4), slice(4, 6),
                           slice(6, 8)):
                    nc.sync.dma_start(wqh[:, sl], wq8h_d[:, sl])
                    nc.sync.dma_start(xhs[0][:, sl], x8h_d[:, 0, sl])
                    nc.sync.dma_start(xls[0][:, sl], x8l_d[:, 0, sl])
                    nc.sync.dma_start(wql[:, sl], wq8l_d[:, sl])
                    nc.sync.dma_start(wkh[:, sl], wk8h_d[:, sl])
                    nc.sync.dma_start(wkl[:, sl], wk8l_d[:, sl])
                    nc.sync.dma_start(wvh[:, sl], wv8h_d[:, sl])
                    nc.sync.dma_start(wvl[:, sl], wv8l_d[:, sl])
                nc.sync.dma_start(idn[:], idn_d[:])
                for n in range(NCH):
                    ps = [aps.tile([128, 512], f32, tag=f"pa{j}", name=f"pa{j}")
                          for j in range(6)]
                    xh, xl = xhs[n % 2], xls[n % 2]
                    if n + 1 < NCH:   # prefetch next chunk
                        for q in range(4):
                            sl = slice(2 * q, 2 * (q + 1))
                            nc.sync.dma_start(xhs[(n + 1) % 2][:, sl],
                                              x8h_d[:, n + 1, sl])
                            nc.sync.dma_start(xls[(n + 1) % 2][:, sl],
                                              x8l_d[:, n + 1, sl])
                    if n == 0:
                        # needed only from the chunk-0 RoPE onward; issued
                        # after the chunk-1 prefetch so that isn't delayed
                        nc.sync.dma_start(cosT[:], cos_d[:])
                        nc.sync.dma_start(sinT[:], sin_d[:])
                        nc.sync.dma_start(shp[:], shp_d[:])
                        # preload the Exp table while Act is idle so the
                        # first phase-B exp doesn't pay LoadActFuncSet
                        warm = at.tile([1, 8], f32, tag="warm", name="warm")
                        nc.scalar.activation(warm[:], shp[0:1, 0:8], Exp)
                    # hi/lo fp8 DoubleRow: exact - (x_lo @ w_lo); contraction
                    # pairs c-tiles (2t, 2t+1) on dim 1. At t=7, v/k groups
                    # stop first so their PSUM->SBUF copies start ASAP.
                    for t in range(8):
                        kv = ((5, wvh, wvl), (4, wkh, wkl))
                        for pj, wh_, wl_ in (kv if t == 7 else ()):
                            for wt, xt in ((wh_, xh), (wh_, xl), (wl_, xh)):
                                nc.tensor.matmul(
                                    ps[pj][:], wt[:, t], xt[:, t],
                                    perf_mode=DR, start=False,
                                    stop=(wt is wl_))
                        for j in range(NH):
                            js = slice(j * 128, (j + 1) * 128)
                            for wt, xt in ((wqh, xh), (wqh, xl), (wql, xh)):
                                nc.tensor.matmul(
                                    ps[j][:], wt[:, t, :, js], xt[:, t],
                                    perf_mode=DR,
                                    start=(t == 0 and xt is xh and wt is wqh),
                                    stop=(t == 7 and wt is wql))
                        if t < 7:
                            for pj, wh_, wl_ in kv:
                                for wt, xt in ((wh_, xh), (wh_, xl), (wl_, xh)):
                                    nc.tensor.matmul(
                                        ps[pj][:], wt[:, t], xt[:, t],
                                        perf_mode=DR,
                                        start=(t == 0 and xt is xh
                                               and wt is wh_),
                                        stop=False)
                    cs = cosT[:, n * 512:(n + 1) * 512]
                    sn = sinT[:, n * 512:(n + 1) * 512]
                    # vf first: PE's next work (transposes) depends on it;
                    # k's RoPE first: phase B's S matmuls depend on kt
                    vf = at.tile([128, 512], bf16, tag="vf", name="vf")
                    nc.scalar.activation(vf[:], ps[5][:], Copy)
                    raws = {}
                    for j in (4, 0, 1, 2, 3):
                        raw = at.tile([128, 512], bf16, tag=f"raw{j}",
                                      name=f"raw{j}", bufs=2)
                        nc.scalar.activation(raw[:], ps[j][:], Copy)
                        # roll along HD via partition-shifted SBUF copies
                        rol = at.tile([128, 512], bf16, tag=f"rol{j}",
                                      name=f"rol{j}", bufs=2)
                        nc.sync.dma_start(rol[1:128, :], raw[0:127, :])
                        nc.sync.dma_start(rol[0:1, :], raw[127:128, :])
                        raws[j] = (raw, rol)
                        if j == 4:
                            for t in range(4):
                                pvt = aps.tile([128, 128], bf16, tag="pvt",
                                               name="pvt")
                                nc.tensor.transpose(
                                    pvt[:], vf[:, t * 128:(t + 1) * 128], idn[:])
                                nc.vector.tensor_copy(vn[n * 4 + t][:], pvt[:])
                    for j in (4, 0, 1, 2, 3):
                        raw, rol = raws[j]
                        t1 = at.tile([128, 512], bf16, tag="t1", name="t1")
                        nc.gpsimd.tensor_tensor(t1[:], raw[:], cs, mult)
                        t2 = at.tile([128, 512], bf16, tag="t2", name="t2")
                        nc.vector.tensor_tensor(t2[:], rol[:], sn, mult)
                        dst = qt[j][n] if j < NH else kt[n]
                        nc.vector.tensor_tensor(dst[:], t1[:], t2[:], add)

            # ------------- Phase B: attention; Phase C: out-projection -------
            # Deferred-work queue: softmax tails and out-projection chunks are
            # emitted interleaved with later heads' S/exp stream so the
            # in-order PE queue never waits on the DVE/Pool reduction chain.
            with tc.tile_pool(name="bexp", bufs=6) as bx, \
                 tc.tile_pool(name="bsacc", bufs=2) as bsa, \
                 tc.tile_pool(name="bsm", bufs=2) as bs, \
                 tc.tile_pool(name="yout", bufs=4) as yp, \
                 tc.tile_pool(name="bpsum", bufs=1, space="PSUM") as bps, \
                 tc.tile_pool(name="cpsum", bufs=2, space="PSUM") as cps:
                workq = []

                def pump(k):
                    for _ in range(min(k, len(workq))):
                        workq.pop(0)()

                from concourse import bass_isa
                for u in range(2):
                    nc.sync.dma_start(woh[u][:], wo8h_d[u])
                    nc.sync.dma_start(wol[u][:], wo8l_d[u])

                def mk_tail(h, qb, saccs, pso):
                    def tail():
                        # all-DVE tree to one [128,512] tile, then a gpsimd
                        # partition all-reduce gives every partition the row
                        # sum -- no PSUM, no ones-matmul, no broadcast matmul
                        l2a = bsa.tile([128, 2, 512], fp16, tag="l2a", name="l2a")
                        nc.vector.tensor_tensor(l2a[:], saccs[0][:], saccs[1][:],
                                                add)
                        l2b = bsa.tile([128, 2, 512], fp16, tag="l2b", name="l2b")
                        nc.vector.tensor_tensor(l2b[:], saccs[2][:], saccs[3][:],
                                                add)
                        l3 = bsa.tile([128, 2, 512], fp16, tag="l3", name="l3")
                        nc.vector.tensor_tensor(l3[:], l2a[:], l2b[:], add)
                        sht = bsa.tile([128, 512], f32, tag="sht", name="sht")
                        nc.vector.tensor_tensor(sht[:], l3[:, 0, :], l3[:, 1, :],
                                                add)
                        sums = bsa.tile([128, 512], f32, tag="sums", name="sums")
                        nc.gpsimd.partition_all_reduce(sums[:], sht[:], 128,
                                                       bass_isa.ReduceOp.add)
                        rec = bs.tile([128, 512], f32r, tag="rec", name="rec")
                        nc.vector.reciprocal(rec[:], sums[:])
                        nf = bs.tile([128, 512], f32, tag="nf", name="nf")
                        nc.vector.tensor_tensor(nf[:], pso[:], rec[:], mult)
                        u, i = h // 2, h % 2
                        # last qb: DVE for the hi/lo split -- it is on the
                        # critical chain into the final out-projection drain
                        eng = nc.vector if qb == NCH - 1 else nc.gpsimd
                        eng.tensor_copy(oth[u][qb][:, i, :], nf[:])
                        eng.tensor_tensor(otl[u][qb][:, i, :], nf[:],
                                          oth[u][qb][:, i, :], sub)
                    return tail

                def mk_cchunk(qb, ti, nn, ysb, last):
                    def cchunk(u=None):
                        # u=None: both head-pairs in one psum group.
                        # u=0/1: split passes (last qb) -- pair-0 matmuls can
                        # run while pair-1's softmax tail is still finishing.
                        ts_ = slice(ti * 128, (ti + 1) * 128)
                        ns_ = slice(nn * 512, (nn + 1) * 512)
                        ys = ysb[:, nn * 512:(nn + 1) * 512]
                        us = (0, 1) if u is None else (u,)
                        psy = cps.tile([128, 512], f32, tag="psy", name="psy")
                        for uu in us:
                            for m, (a, w) in enumerate(
                                    ((oth, woh), (oth, wol), (otl, woh))):
                                nc.tensor.matmul(
                                    psy[:], a[uu][qb][:, :, ts_], w[uu][:, :, ns_],
                                    perf_mode=DR,
                                    start=(uu == us[0] and m == 0),
                                    stop=(uu == us[-1] and m == 2))
                        if u == 0:
                            nc.vector.tensor_copy(ys, psy[:])
                            return
                        if u == 1:
                            nc.vector.tensor_tensor(ys, ys, psy[:], add)
                        elif last:
                            nc.scalar.activation(ys, psy[:], Copy)
                        else:
                            nc.vector.tensor_copy(ys, psy[:])
                        qtile = qb * 4 + ti
                        if last:
                            # final tile: store per-slice so the tail DMA is
                            # small and the drain starts sooner
                            nc.sync.dma_start(
                                y_d[qtile * 128:(qtile + 1) * 128,
                                    nn * 512:(nn + 1) * 512], ys)
                        elif nn == NCH - 1:
                            nc.sync.dma_start(
                                y_d[qtile * 128:(qtile + 1) * 128, :], ysb[:])
                    return cchunk

                for qb in range(NCH):
                    for h in range(NH):
                        pso = bps.tile([128, 512], f32, tag=f"pso{(qb * 4 + h) % 2}",
                                       name="pso")
                        saccs, es_tiles = [], []

                        def consume(pt):
                            # AV + level-1 row-sum for es_tiles[pt], one step
                            # behind the S/exp stream so PE never waits on Act
                            es = es_tiles[pt]
                            nc.tensor.matmul(pso[:], vn[2 * pt][:], es[:, 0, :],
                                             start=(pt == 0), stop=False)
                            nc.tensor.matmul(pso[:], vn[2 * pt + 1][:],
                                             es[:, 1, :],
                                             start=False, stop=(pt == 7))
                            if pt % 2 == 1:
                                sa = bsa.tile([128, 2, 512], fp16,
                                              tag=f"sa{pt // 2}",
                                              name=f"sa{pt // 2}")
                                nc.vector.tensor_tensor(sa[:], es_tiles[pt - 1][:],
                                                        es[:], add)
                                saccs.append(sa)

                        for pt in range(8):
                            pss = bps.tile([128, 2, 512], f32,
                                           tag=f"pss{pt % 2}", name=f"pss{pt % 2}")
                            for half in range(2):
                                k = 2 * pt + half
                                nc.tensor.matmul(
                                    pss[:, half, :],
                                    kt[k // 4][:, (k % 4) * 128:(k % 4 + 1) * 128],
                                    qt[h][qb][:], start=True, stop=True)
                            es = bx.tile([128, 2, 512], fp16, tag="es", name="es")
                            nc.scalar.activation(es[:], pss[:], Exp, scale=SCALE)
                            es_tiles.append(es)
                            if pt > 0:
                                consume(pt - 1)
                            pump(1)
                        consume(7)
                        workq.append(mk_tail(h, qb, saccs, pso))
                    # let the last head's tail chain (~5us of DVE/Pool
                    # latency) finish before the first C chunk needs its ot
                    workq.extend([lambda: None] * 8)
                    for ti in range(4):
                        ysb = yp.tile([128, L], f32, tag="ysb", name="ysb")
                        for nn in range(NCH):
                            workq.append(mk_cchunk(qb, ti, nn, ysb,
                                                   qb == NCH - 1))
                pump(len(workq))

    nc.compile()
    return nc


def _host_inputs(x, Wq, Wk, Wv, Wo):
    inv = 1.0 / (BASE ** (np.arange(0, HD, 2, dtype=np.float32) / HD))
    pos = np.arange(L, dtype=np.float32)
    fr = pos[:, None] * inv[None, :]
    emb = np.concatenate([fr, fr], axis=1)            # [L, HD]
    cosT = np.ascontiguousarray(np.cos(emb).T).astype(BF16)
    sinT = np.ascontiguousarray(np.sin(emb).T).astype(BF16)
    shp = np.zeros((HD, HD), np.float32)
    shp[(np.arange(HD) - 1) % HD, np.arange(HD)] = 1.0
    idn = np.eye(128, dtype=np.float32)

    FP8 = ml_dtypes.float8_e4m3
    FP8E5 = ml_dtypes.float8_e5m2

    def hilo(a):
        hi = a.astype(FP8)
        lo = (a - hi.astype(np.float32)).astype(FP8E5)
        return hi, lo

    maps = []
    for c in range(8):
        b, g = c // 4, c % 4
        xT = x[b].T                                    # [D, L]
        # [p, n, t, i, m] = xT[256t+128i+p, 512n+m]
        xa = xT.reshape(8, 2, 128, NCH, 512).transpose(2, 3, 0, 1, 4)
        x8h, x8l = hilo(np.ascontiguousarray(xa))
        # weights: [p, t, i, m] = 32*W[256t+128i+p, m]
        wq = (Wq[:, g * NH * HD:(g + 1) * NH * HD] * WS)
        wq8h, wq8l = hilo(np.ascontiguousarray(
            wq.reshape(8, 2, 128, NH * HD).transpose(2, 0, 1, 3)))
        wk = (Wk[:, g * HD:(g + 1) * HD] * WS)
        wk8h, wk8l = hilo(np.ascontiguousarray(
            wk.reshape(8, 2, 128, HD).transpose(2, 0, 1, 3)))
        wv = (Wv[:, g * HD:(g + 1) * HD] * WS)
        wv8h, wv8l = hilo(np.ascontiguousarray(
            wv.reshape(8, 2, 128, HD).transpose(2, 0, 1, 3)))
        # wo pairs: [u, p, i, m] = 32*Wo[g*512 + (2u+i)*128 + p, m]
        wo = (Wo[g * NH * HD:(g + 1) * NH * HD, :] * WS)
        wo8h, wo8l = hilo(np.ascontiguousarray(
            wo.reshape(2, 2, 128, D).transpose(0, 2, 1, 3)))
        maps.append({
            "x8h": x8h, "x8l": x8l,
            "wq8h": wq8h, "wq8l": wq8l, "wk8h": wk8h, "wk8l": wk8l,
            "wv8h": wv8h, "wv8l": wv8l, "wo8h": wo8h, "wo8l": wo8l,
            "cosT": cosT, "sinT": sinT,
            "shiftP": shp.astype(BF16), "ident": idn.astype(BF16),
        })
    return maps


def _run(inputs, trace=False):
    global _compiled
    from concourse.bass_utils import run_bass_kernel_spmd
    if _compiled is None:
        _compiled = _build()
    maps = _host_inputs(inputs["x"], inputs["Wq"], inputs["Wk"],
                        inputs["Wv"], inputs["Wo"])
    res = run_bass_kernel_spmd(_compiled, maps, list(range(8)), trace=trace)
    y = np.empty((B, L, D), np.float32)
    for b in range(B):
        y[b] = res.results[b * 4]["y"]
        for g in range(1, 4):
            y[b] += res.results[b * 4 + g]["y"]
    y *= 1.0 / (WS * WS)   # v and wo each carry the x32 host prescale
    return y, res


def kernel(**inputs):
    x = np.asarray(inputs["x"], np.float32)
    y, _ = _run({"x": x,
                 "Wq": np.asarray(inputs["Wq"], np.float32),
                 "Wk": np.asarray(inputs["Wk"], np.float32),
                 "Wv": np.asarray(inputs["Wv"], np.float32),
                 "Wo": np.asarray(inputs["Wo"], np.float32)})
    return y
